# revision 5
# baseline (speedup 1.0000x reference)
"""Sharded attention-energy + softmax kernel for 8 trn2 NeuronCores.

Math: energies = (E @ W.T + b) @ hidden = E @ (hidden @ W) + (b.hidden)
The (b.hidden) term is a constant shift of all logits, which softmax
cancels exactly, so the device only computes e = E @ u with
u = hidden @ W (tiny host-side matvec), then a numerically-stable
sharded softmax: each core emits exp(e - rowmax) plus per-partition
max/sum stats; the host performs the standard two-pass softmax merge.

Sharding: encoder_outputs [32768, 1024] split along seq into 8 shards
of [4096, 1024] (one per core); u replicated (pre-broadcast to 128
partitions on the host so it loads via a plain contiguous HWDGE DMA).

Per core the device streams the 16.8 MB shard through SBUF and fuses
multiply+reduce in one DVE pass per 1024-wide row (affine_mul_reduce).
Loads alternate between the two HWDGE rings (sync/scalar). The Tile
exit barrier is patched to clear only the semaphores the kernel
actually touched (instead of the whole sem file) and to skip the
redundant DMA-reset drain - both are pure NEFF-epilogue overhead.
"""

import numpy as np

H = 1024
S = 32768
NCORES = 8
SSH = S // NCORES          # 4096 seq rows per core
P = 128                    # SBUF partitions
NCOL = SSH // P            # 32 energy columns per core
# column-group sizes per DMA: small first tiles for fast pipeline ramp
QS = [1, 1, 2, 4, 4, 4, 4, 4, 4, 4]
assert sum(QS) == NCOL

_nc = None
_patched = False


def _patch_tile_exit():
    """Make the Tile exit epilogue cheap: clear only semaphores that any
    instruction actually waits on / updates (instead of the entire
    semaphore file, whose per-sem clearing serializes ~6us on the slow
    Tensor sequencer), and skip the redundant dma_reset drain (the sync
    drain + barrier that precede it already guarantee DMA completion)."""
    global _patched
    if _patched:
        return
    _patched = True
    from concourse.bass import Bass, SemaphoreHandle, compact_to_ranges

    def clear_and_free_semaphores(self, sems):
        if not sems:
            return
        sem_nums = [
            sem.num if isinstance(sem, SemaphoreHandle) else sem for sem in sems
        ]
        used = set()
        for inst in self.inst_map.values():
            si = getattr(inst, "sync_info", None)
            if si is None:
                continue
            for lst in (si.on_wait or [], si.on_update or []):
                for ent in lst:
                    sid = getattr(ent, "id", None)
                    if sid is not None:
                        used.add(int(sid))
        to_clear = sorted(set(sem_nums) & used)
        for sem_range in compact_to_ranges(to_clear):
            self.gpsimd.sem_clear(sem_range)
        self._state.prepend_free_semaphores(sem_nums)
        for poison_set in self._tile_sem_poison_stack:
            poison_set.update(sem_nums)

    Bass.clear_and_free_semaphores = clear_and_free_semaphores


def _build():
    import concourse.bacc as bacc
    import concourse.tile as tile
    from concourse import mybir

    _patch_tile_exit()

    f32 = mybir.dt.float32
    nc = bacc.Bacc()

    enc = nc.declare_dram_parameter("enc", [SSH, H], f32, isOutput=False)
    u = nc.declare_dram_parameter("u", [P, H], f32, isOutput=False)
    # out[:, :NCOL] = exp(e - m) ; out[:, NCOL] = m ; out[:, NCOL+1] = sum
    out = nc.declare_dram_parameter("out", [P, NCOL + 2], f32, isOutput=True)

    enc_flat = enc[:]  # [SSH, H]

    with tile.TileContext(nc) as tc:
        with (
            tc.tile_pool(name="singles", bufs=1) as singles,
            tc.tile_pool(name="loads", bufs=5) as loads,
        ):
            u_b = singles.tile([P, H], f32)
            nc.sync.dma_start(out=u_b, in_=u[:])

            e_sb = singles.tile([P, NCOL], f32)
            dummy = singles.tile([P, 1], f32)

            col = 0
            for n, q in enumerate(QS):
                # rows [col*P, (col+q)*P) viewed as [P, q, H]:
                # row col*P + j*P + p -> partition p, block j
                src = enc_flat[col * P : (col + q) * P, :].rearrange(
                    "(j p) h -> p j h", p=P
                )
                t = loads.tile([P, q, H], f32, tag="loads")
                eng = nc.sync if n % 2 == 0 else nc.scalar
                eng.dma_start(out=t, in_=src)
                for j in range(q):
                    nc.vector.affine_mul_reduce(
                        out=dummy.broadcast_to([P, H]),
                        accum_out=e_sb[:, col + j : col + j + 1],
                        in0=t[:, j, :],
                        in1=u_b,
                        scale=1.0,
                        bias=0.0,
                    )
                col += q

            combo = singles.tile([P, NCOL + 2], f32)
            neg = singles.tile([P, 1], f32)
            nc.vector.tensor_reduce(
                out=combo[:, NCOL : NCOL + 1],
                in_=e_sb,
                axis=mybir.AxisListType.X,
                op=mybir.AluOpType.max,
            )
            nc.vector.tensor_scalar_mul(neg, combo[:, NCOL : NCOL + 1], -1.0)
            nc.scalar.activation(
                out=combo[:, :NCOL],
                in_=e_sb,
                func=mybir.ActivationFunctionType.Exp,
                bias=neg,
                scale=1.0,
                accum_out=combo[:, NCOL + 1 : NCOL + 2],
            )
            nc.sync.dma_start(out=out[:], in_=combo)
    nc.finalize()
    return nc


# Set by a driver (e.g. test.py) to capture a profiled run.
PROFILE = False
LAST_RESULT = None


def kernel(hidden, encoder_outputs, W, b):
    global _nc, LAST_RESULT
    from concourse.bass_utils import run_bass_kernel_spmd

    if _nc is None:
        _nc = _build()

    hidden = np.asarray(hidden)
    encoder_outputs = np.ascontiguousarray(np.asarray(encoder_outputs))
    W = np.asarray(W)

    u = (hidden.astype(np.float64) @ W.astype(np.float64)).astype(np.float32)
    u_rep = np.ascontiguousarray(np.broadcast_to(u, (P, H)))

    in_maps = [
        {"enc": encoder_outputs[i * SSH : (i + 1) * SSH], "u": u_rep}
        for i in range(NCORES)
    ]
    res = run_bass_kernel_spmd(
        _nc, in_maps, core_ids=list(range(NCORES)), trace=PROFILE
    )
    if PROFILE:
        LAST_RESULT = res

    outs = np.stack([r["out"] for r in res.results])  # [8, 128, 34]
    p_exp = outs[:, :, :NCOL].astype(np.float64)      # [8, 128, 32]
    m = outs[:, :, NCOL].astype(np.float64)           # [8, 128]
    ssum = outs[:, :, NCOL + 1].astype(np.float64)    # [8, 128]

    M = m.max()
    scale = np.exp(m - M)                             # [8, 128]
    Z = (ssum * scale).sum()
    attn = p_exp * (scale / Z)[:, :, None]            # [8, 128, 32]
    # element (core i, partition p, col c) is seq index i*SSH + c*P + p
    full = attn.transpose(0, 2, 1).reshape(-1).astype(np.float32)
    return full.reshape(1, 1, S)
